# revision 2
# baseline (speedup 1.0000x reference)
"""GATv2 encoder (nn_Encoder_83614423318750) — v2 8-core TRN2 Bass kernel.

Layout strategy (per core, SPMD; nodes block-sharded, degree-sorted):

  A : hT = (x @ ae_w.T + ae_b).T computed directly in [C, n] layout via
      PE (lhsT = ae_w chunks, rhs = x^T chunks) -> hT_store (SBUF, bf16).
      xr = h @ wr.T node-major via PE (lhsT = hT slice, rhs = wr.T).
  AG: AllGather hT shards -> hT_full [8*64, NPAD] (DRAM).
  A2: xl_full[n, hc] = h @ wl.T for ALL nodes, via PE (lhsT = hT_full
      slice, rhs = wl.T) -> DRAM bf16 (gather source).
  B : per 128-dst-node tile, per slot-subgroup (8 slots = 8 chunks of
      128 edges, chunk = slot s of all 128 dst nodes):
        * dma_gather  g   [128d, 8s, 256c]   (messages, edge-major)
        * dma_gather  gT  [128c, 2, 8s*128d] (transpose=True: score side)
        * pT = prelu(gT + xrT)               (DVE add + ScalarE Prelu)
        * sT[4h, e] = attT.T @ pT            (PE, PSUM-accumulated blocks)
        * exT = exp(sT)  (no segment-max: scores are O(5), fp32-exp safe)
        * ex[128d, s, 4h] via PE transpose;  ex *= valid (kills padding)
        * rhs = [ g * ex_bc | ex ]  [128, 260]
        * agg_ps[128, 260] += I.T @ rhs      (identity matmul: segment-sum
          of messages AND softmax denominators in one PSUM accumulation)
      epilogue: alpha-normalize by 1/ssum, z = relu(lin(agg)+lin_b+gat_b
      (folded)), r = z + h, LayerNorm -> out (fp32).

Padding slots carry valid=0 -> ex=0 -> zero contribution. Gather tables
int16 split at TSPLIT=32768 rows (A/B) as in the original kernel.
"""

import numpy as np
from contextlib import ExitStack

import ml_dtypes

import concourse.bass as bass
import concourse.bacc as bacc
import concourse.tile as tile
from concourse import mybir, bass_utils
from concourse.masks import make_identity

F32 = mybir.dt.float32
BF16 = mybir.dt.bfloat16
I16 = mybir.dt.int16
BFNP = ml_dtypes.bfloat16

FULL_CFG = dict(N=50000, IC=2000, C=64, H=4, E=800000, NCORES=8, TSPLIT=32768)

NEG_SLOPE = 0.2
LN_EPS = 1e-12
SM_EPS = 1e-9
SG = 8      # slots (=128-edge chunks) per subgroup
A2G = 4     # node-chunks per xl write group

_PROGRAM_CACHE = {}


# --------------------------------------------------------------------------
# host-side preprocessing (slot tables identical in structure to v1)
# --------------------------------------------------------------------------

def _preprocess(x, edge_index, cfg):
    N, IC, C, H, NCORES = cfg["N"], cfg["IC"], cfg["C"], cfg["H"], cfg["NCORES"]
    TSPLIT = cfg["TSPLIT"]
    NSH = N // NCORES
    NT = (NSH + 127) // 128
    NPAD = NT * 128
    NTOT = NCORES * NPAD
    ICP = ((IC + 127) // 128) * 128
    KT = ICP // 128

    src = np.asarray(edge_index[0], dtype=np.int64)
    dst = np.asarray(edge_index[1], dtype=np.int64)

    order = np.argsort(dst, kind="stable")
    src_s = src[order].astype(np.int64)
    counts = np.bincount(dst, minlength=N)
    starts = np.zeros(N, np.int64)
    starts[1:] = np.cumsum(counts)[:-1]
    deg = counts + 1  # + self loop

    perms = []
    for k in range(NCORES):
        degk = deg[k * NSH:(k + 1) * NSH]
        perms.append(np.argsort(-degk, kind="stable"))

    gmap = np.zeros(N, np.int64)
    for k in range(NCORES):
        gmap[k * NSH + perms[k]] = k * NPAD + np.arange(NSH)

    KMAXDEG = int(deg.max())
    jj = np.arange(KMAXDEG)[None, :]

    nA_all = np.zeros((NCORES, NPAD), np.int64)
    nB_all = np.zeros((NCORES, NPAD), np.int64)
    EMg_all = []
    for k in range(NCORES):
        perm = perms[k]
        vglob = k * NSH + perm
        dpn = np.zeros(NPAD, np.int64)
        dpn[:NSH] = deg[vglob]
        st = np.zeros(NPAD, np.int64)
        st[:NSH] = starts[vglob]
        vg = np.zeros(NPAD, np.int64)
        vg[:NSH] = vglob

        valid = jj < dpn[:, None]
        is_self = jj == (dpn - 1)[:, None]
        eidx = np.minimum(st[:, None] + jj, len(src_s) - 1)
        esrc = np.where(valid & ~is_self, src_s[eidx], vg[:, None])
        EMg = np.where(valid, gmap[esrc], 0)
        isA = (EMg < TSPLIT) & valid
        keys = np.where(valid, np.where(isA, 0, 1), 2)
        ordr = np.argsort(keys, axis=1, kind="stable")
        EMg_sorted = np.take_along_axis(EMg, ordr, axis=1)
        nA = isA.sum(1)
        nB = valid.sum(1) - nA
        nA_all[k], nB_all[k] = nA, nB
        EMg_all.append(EMg_sorted)

    KA = np.zeros(NT, np.int64)
    KB = np.zeros(NT, np.int64)
    for t in range(NT):
        sl = slice(t * 128, (t + 1) * 128)
        KA[t] = max(1, int(nA_all[:, sl].max()))
        KB[t] = int(nB_all[:, sl].max())
    K = KA + KB

    def pack_idx16(vals):                            # [128, Kg] -> [128, 8*Kg]
        L = vals.shape[1] * 128
        flat = vals.T.reshape(-1)                    # flat[j*128+p] = vals[p,j]
        idx16 = flat.reshape(L // 16, 16).T.astype(np.int16)
        return np.tile(idx16, (8, 1))

    coreinfo = []
    for k in range(NCORES):
        EMg_sorted = EMg_all[k]
        nA, nB = nA_all[k], nB_all[k]
        idxa_parts, idxb_parts, valid_parts = [], [], []
        for t in range(NT):
            sl = slice(t * 128, (t + 1) * 128)
            ka, kb = int(KA[t]), int(KB[t])
            em = EMg_sorted[sl]
            na = nA[sl][:, None]
            nb = nB[sl][:, None]
            ja = np.arange(ka)[None, :]
            srcA = np.where(ja < na, em[:, :ka], 0)
            idxa_parts.append(pack_idx16(srcA))
            if kb > 0:
                jb = np.arange(kb)[None, :]
                gidx = np.minimum(na + jb, EMg_sorted.shape[1] - 1)
                srcB = np.where(jb < nb,
                                np.take_along_axis(em, gidx, axis=1) - TSPLIT, 0)
                srcB = np.maximum(srcB, 0)
                idxb_parts.append(pack_idx16(srcB))
            m = np.zeros((128, ka + kb), np.float32)
            m[:, :ka][ja < na] = 1.0
            if kb > 0:
                m[:, ka:][jb < nb] = 1.0
            valid_parts.append(m)
        coreinfo.append(dict(
            idxa=np.concatenate(idxa_parts, axis=1),
            idxb=(np.concatenate(idxb_parts, axis=1) if idxb_parts
                  else np.zeros((128, 0), np.int16)),
            valid=np.concatenate(valid_parts, axis=1).astype(BFNP),
        ))

    # x shards: permuted, padded, transposed, tiled, bf16
    xtts = []
    for k in range(NCORES):
        xs = np.zeros((NPAD, ICP), np.float32)
        xs[:NSH, :IC] = x[k * NSH:(k + 1) * NSH][perms[k]]
        xtt = xs.reshape(NT, 128, KT, 128).transpose(0, 3, 2, 1)
        xtts.append(np.ascontiguousarray(xtt).astype(BFNP))

    meta = dict(NSH=NSH, NT=NT, NPAD=NPAD, NTOT=NTOT, ICP=ICP, KT=KT,
                KA=tuple(int(v) for v in KA), KB=tuple(int(v) for v in KB),
                K=tuple(int(v) for v in K))
    return meta, perms, coreinfo, xtts


# --------------------------------------------------------------------------
# device program
# --------------------------------------------------------------------------

def _build_program(cfg, meta):
    C, H, NCORES = cfg["C"], cfg["H"], cfg["NCORES"]
    HC = H * C
    NT, NPAD, NTOT = meta["NT"], meta["NPAD"], meta["NTOT"]
    TSPLIT = min(cfg["TSPLIT"], NTOT)
    KT = meta["KT"]
    KA, KB, K = meta["KA"], meta["KB"], meta["K"]
    SUMK = sum(K)
    SUMIA = sum(8 * ka for ka in KA)
    SUMIB = sum(8 * kb for kb in KB)
    W = HC + H  # 260: [amsg | ex]

    nc = bacc.Bacc("TRN2", target_bir_lowering=False, debug=False,
                   num_devices=NCORES)

    # ---- external I/O ----
    xtt = nc.dram_tensor("xtt", [NT, 128, KT, 128], BF16, kind="ExternalInput")
    aewt = nc.dram_tensor("aewt", [128, KT, C], BF16, kind="ExternalInput")
    wlt = nc.dram_tensor("wlt", [C, HC], BF16, kind="ExternalInput")
    wrt = nc.dram_tensor("wrt", [C, HC], BF16, kind="ExternalInput")
    linwt = nc.dram_tensor("linwt", [128, 2, C], BF16, kind="ExternalInput")
    attT = nc.dram_tensor("attT", [128, 2, H], BF16, kind="ExternalInput")
    aeb = nc.dram_tensor("aeb", [C, 1], F32, kind="ExternalInput")
    linb2 = nc.dram_tensor("linb2", [C, 1], F32, kind="ExternalInput")
    lnw = nc.dram_tensor("lnw", [C], F32, kind="ExternalInput")
    lnb = nc.dram_tensor("lnb", [C], F32, kind="ExternalInput")
    idxa_d = nc.dram_tensor("idxa", [128, SUMIA], I16, kind="ExternalInput")
    idxb_d = nc.dram_tensor("idxb", [128, max(SUMIB, 1)], I16,
                            kind="ExternalInput")
    valid_d = nc.dram_tensor("valid", [128, SUMK], BF16, kind="ExternalInput")
    out_d = nc.dram_tensor("out", [NPAD, C], F32, kind="ExternalOutput")

    def bc_row(t, n):  # DRAM [n] -> broadcast AP [128, n]
        return bass.AP(tensor=t[:].tensor, offset=0, ap=[[0, 128], [1, n]])

    with tile.TileContext(nc) as tc:
        with ExitStack() as ctx:
            dram = ctx.enter_context(tc.tile_pool(name="dram", bufs=1,
                                                  space="DRAM"))
            hT_shard_d = dram.tile([C, NPAD], BF16)
            hT_full = dram.tile([NCORES * C, NPAD], BF16, addr_space="Shared")
            xr_d = dram.tile([NPAD, HC], BF16)
            xl_full = dram.tile([NTOT, HC], BF16)

            # ---- persistent SBUF ----
            consts = ctx.enter_context(tc.tile_pool(name="consts", bufs=1))
            identb = consts.tile([128, 128], BF16)
            make_identity(nc, identb[:])
            aewt_sb = consts.tile([128, KT, C], BF16)
            nc.sync.dma_start(out=aewt_sb[:], in_=aewt[:])
            wlt_sb = consts.tile([C, HC], BF16)
            nc.sync.dma_start(out=wlt_sb[:], in_=wlt[:])
            wrt_sb = consts.tile([C, HC], BF16)
            nc.sync.dma_start(out=wrt_sb[:], in_=wrt[:])
            linwt_sb = consts.tile([128, 2, C], BF16)
            nc.sync.dma_start(out=linwt_sb[:], in_=linwt[:])
            attT_sb = consts.tile([128, 2, H], BF16)
            nc.sync.dma_start(out=attT_sb[:], in_=attT[:])
            aeb_sb = consts.tile([C, 1], F32)
            nc.sync.dma_start(out=aeb_sb[:], in_=aeb[:])
            linb2_sb = consts.tile([C, 1], F32)
            nc.sync.dma_start(out=linb2_sb[:], in_=linb2[:])
            lnw_rep = consts.tile([128, C], F32)
            nc.sync.dma_start(out=lnw_rep[:], in_=bc_row(lnw, C))
            lnb_rep = consts.tile([128, C], F32)
            nc.sync.dma_start(out=lnb_rep[:], in_=bc_row(lnb, C))
            eps_col = consts.tile([128, 1], F32)
            nc.vector.memset(eps_col[:], LN_EPS)
            zeros_c = consts.tile([C, 128], F32)
            nc.vector.memset(zeros_c[:], 0.0)

            hT_store = consts.tile([C, NT, 128], BF16)

            idx_arena = consts.tile([128, SUMIA + max(SUMIB, 1)], I16)
            nc.sync.dma_start(out=idx_arena[:, :SUMIA], in_=idxa_d[:])
            if SUMIB > 0:
                nc.sync.dma_start(out=idx_arena[:, SUMIA:], in_=idxb_d[:])
            valid_arena = consts.tile([128, SUMK], BF16)
            nc.sync.dma_start(out=valid_arena[:], in_=valid_d[:])

            # ================= phase A =================
            with ExitStack() as actx:
                xsl_p = actx.enter_context(tc.tile_pool(name="xsl", bufs=2))
                ps_h = actx.enter_context(
                    tc.tile_pool(name="ps_h", bufs=2, space="PSUM"))
                ps_xr = actx.enter_context(
                    tc.tile_pool(name="ps_xr", bufs=2, space="PSUM"))
                sb_a = actx.enter_context(tc.tile_pool(name="sb_a", bufs=2))

                for t in range(NT):
                    xslab = xsl_p.tile([128, KT, 128], BF16, tag="xslab")
                    nc.sync.dma_start(out=xslab[:], in_=xtt[t])
                    hT_ps = ps_h.tile([C, 128], F32, tag="hT_ps")
                    for kk in range(KT):
                        nc.tensor.matmul(out=hT_ps[:],
                                         lhsT=aewt_sb[:, kk, :],
                                         rhs=xslab[:, kk, :],
                                         start=(kk == 0), stop=(kk == KT - 1))
                    # hT = hT_ps + aeb (per-partition) -> bf16
                    aeb_b = aeb_sb[:].to_broadcast([C, 128])
                    nc.vector.tensor_tensor(out=hT_store[:, t, :],
                                            in0=hT_ps[:], in1=aeb_b,
                                            op=mybir.AluOpType.add)
                    xr_ps = ps_xr.tile([128, HC], F32, tag="xr_ps")
                    nc.tensor.matmul(out=xr_ps[:], lhsT=hT_store[:, t, :],
                                     rhs=wrt_sb[:], start=True, stop=True)
                    xr_sb = sb_a.tile([128, HC], BF16, tag="xr_sb")
                    nc.vector.tensor_copy(out=xr_sb[:], in_=xr_ps[:])
                    nc.sync.dma_start(out=xr_d[t * 128:(t + 1) * 128, :],
                                      in_=xr_sb[:])
                nc.sync.dma_start(
                    out=hT_shard_d[:],
                    in_=hT_store[:].rearrange("c t p -> c (t p)"))

            # ================= AllGather =================
            nc.gpsimd.collective_compute(
                "AllGather", mybir.AluOpType.bypass,
                ins=[hT_shard_d[:].opt()], outs=[hT_full[:].opt()],
                replica_groups=[list(range(NCORES))])

            # ================= phase A2: xl_full build =================
            with ExitStack() as actx:
                htf_p = actx.enter_context(tc.tile_pool(name="htf", bufs=2))
                ps_xl = actx.enter_context(
                    tc.tile_pool(name="ps_xl", bufs=2, space="PSUM"))
                sb_xl = actx.enter_context(tc.tile_pool(name="sb_xl", bufs=2))
                for cb in range(NCORES):
                    hTf = htf_p.tile([C, NPAD], BF16, tag="hTf")
                    nc.sync.dma_start(out=hTf[:],
                                      in_=hT_full[cb * C:(cb + 1) * C, :])
                    for g0 in range(0, NT, A2G):
                        gn = min(A2G, NT - g0)
                        xl_sb = sb_xl.tile([128, A2G, HC], BF16, tag="xl_sb")
                        for i in range(gn):
                            lc = g0 + i
                            xl_ps = ps_xl.tile([128, HC], F32, tag="xl_ps")
                            nc.tensor.matmul(
                                out=xl_ps[:],
                                lhsT=hTf[:, lc * 128:(lc + 1) * 128],
                                rhs=wlt_sb[:], start=True, stop=True)
                            nc.vector.tensor_copy(out=xl_sb[:, i, :],
                                                  in_=xl_ps[:])
                        r0 = cb * NPAD + g0 * 128
                        nc.sync.dma_start(
                            out=xl_full[r0:r0 + gn * 128, :].rearrange(
                                "(t p) c -> p t c", p=128),
                            in_=xl_sb[:, :gn, :])

            # ================= phase B =================
            with ExitStack() as bctx:
                g_pool = bctx.enter_context(tc.tile_pool(name="g", bufs=3))
                gt_pool = bctx.enter_context(tc.tile_pool(name="gt", bufs=3))
                r_pool = bctx.enter_context(tc.tile_pool(name="rhs", bufs=3))
                ext_pool = bctx.enter_context(tc.tile_pool(name="ext", bufs=3))
                xr_pool = bctx.enter_context(tc.tile_pool(name="xrl", bufs=2))
                epi_pool = bctx.enter_context(tc.tile_pool(name="epi", bufs=2))
                ln_pool = bctx.enter_context(tc.tile_pool(name="ln", bufs=2))
                ps_sT = bctx.enter_context(
                    tc.tile_pool(name="ps_sT", bufs=1, space="PSUM"))
                ps_ex = bctx.enter_context(
                    tc.tile_pool(name="ps_ex", bufs=2, space="PSUM"))
                ps_agg = bctx.enter_context(
                    tc.tile_pool(name="ps_agg", bufs=2, space="PSUM"))
                ps_tr = bctx.enter_context(
                    tc.tile_pool(name="ps_tr", bufs=2, space="PSUM"))

                ioffA = 0
                ioffB = SUMIA
                koff = 0
                for t in range(NT):
                    ka, kb, kt_ = KA[t], KB[t], K[t]

                    xr_t = xr_pool.tile([128, HC], BF16, tag="xr_t")
                    nc.sync.dma_start(out=xr_t[:],
                                      in_=xr_d[t * 128:(t + 1) * 128, :])
                    xrT_ps = ps_tr.tile([128, 2, 128], BF16, tag="trT")
                    for blk in range(2):
                        nc.tensor.transpose(
                            out=xrT_ps[:, blk, :],
                            in_=xr_t[:, blk * 128:(blk + 1) * 128],
                            identity=identb[:])
                    xrT_sb = xr_pool.tile([128, 2, 128], BF16, tag="xrT_sb")
                    nc.vector.tensor_copy(out=xrT_sb[:], in_=xrT_ps[:])

                    agg_ps = ps_agg.tile([128, W], F32, tag="agg_ps")

                    # subgroup list: (source-half, slot0, bn, idx column off)
                    subs = []
                    for s0 in range(0, ka, SG):
                        subs.append((0, s0, min(SG, ka - s0),
                                     ioffA + 8 * s0))
                    for s0 in range(0, kb, SG):
                        subs.append((1, ka + s0, min(SG, kb - s0),
                                     ioffB + 8 * s0))
                    ioffA += 8 * ka
                    ioffB += 8 * kb

                    nsub = len(subs)
                    for si, (half, s0, bn, ioff) in enumerate(subs):
                        ne = bn * 128
                        idx = idx_arena[:, ioff:ioff + 8 * bn]
                        src = (xl_full[0:TSPLIT, :] if half == 0
                               else xl_full[TSPLIT:NTOT, :])
                        g_sb = g_pool.tile([128, SG, HC], BF16, tag="g")
                        nc.gpsimd.dma_gather(
                            g_sb[:, :bn, :], src, idx, ne, ne, HC)
                        # transposed side in half-subgroups of <=4 chunks
                        # (transpose dma_gather breaks above ~768 idxs)
                        sT_ps = ps_sT.tile([H, SG * 128], F32, tag="sT_ps")
                        for c0 in range(0, bn, 4):
                            cn = min(4, bn - c0)
                            hne = cn * 128
                            gt_flat = gt_pool.tile([128, 2 * 4 * 128], BF16,
                                                   tag="gT")
                            gT = gt_flat[:, :2 * hne].rearrange(
                                "p (b e) -> p b e", b=2)
                            nc.gpsimd.dma_gather(
                                gT, src, idx_arena[:, ioff + 8 * c0:
                                                   ioff + 8 * (c0 + cn)],
                                hne, hne, HC, transpose=True)
                            gT4 = gt_flat[:, :2 * hne].rearrange(
                                "p (b k e) -> p b k e", b=2, k=cn)
                            xrT_b = xrT_sb[:][:, :, None, :].to_broadcast(
                                [128, 2, cn, 128])
                            nc.vector.tensor_tensor(out=gT4, in0=gT4,
                                                    in1=xrT_b,
                                                    op=mybir.AluOpType.add)
                            nc.scalar.activation(
                                out=gt_flat[:, :2 * hne],
                                in_=gt_flat[:, :2 * hne],
                                func=mybir.ActivationFunctionType.Prelu,
                                alpha=NEG_SLOPE)
                            for blk in range(2):
                                nc.tensor.matmul(
                                    out=sT_ps[:, c0 * 128:c0 * 128 + hne],
                                    lhsT=attT_sb[:, blk, :],
                                    rhs=gt_flat[:, blk * hne:
                                                blk * hne + hne],
                                    start=(blk == 0), stop=(blk == 1))
                        exT = ext_pool.tile([H, SG * 128], BF16, tag="exT")
                        nc.scalar.activation(
                            out=exT[:, :ne], in_=sT_ps[:, :ne],
                            func=mybir.ActivationFunctionType.Exp)
                        ex_ps = ps_ex.tile([128, SG, H], BF16, tag="ex_ps")
                        for j in range(bn):
                            nc.tensor.transpose(
                                out=ex_ps[:, j, :],
                                in_=exT[:, j * 128:(j + 1) * 128],
                                identity=identb[:H, :H])
                        rhs_sb = r_pool.tile([128, SG, W], BF16, tag="rhs")
                        vb = valid_arena[:, koff + s0:koff + s0 + bn]
                        nc.vector.tensor_tensor(
                            out=rhs_sb[:, :bn, HC:W],
                            in0=ex_ps[:, :bn, :],
                            in1=vb[:, :, None].to_broadcast([128, bn, H]),
                            op=mybir.AluOpType.mult)
                        g4 = g_sb[:, :bn, :].rearrange(
                            "p k (h c) -> p k h c", c=C)
                        ex_b = rhs_sb[:, :bn, HC:W][:, :, :, None] \
                            .to_broadcast([128, bn, H, C])
                        nc.vector.tensor_tensor(
                            out=rhs_sb[:, :bn, 0:HC].rearrange(
                                "p k (h c) -> p k h c", c=C),
                            in0=g4, in1=ex_b, op=mybir.AluOpType.mult)
                        for j in range(bn):
                            nc.tensor.matmul(
                                out=agg_ps[:],
                                lhsT=identb[:],
                                rhs=rhs_sb[:, j, :],
                                start=(si == 0 and j == 0),
                                stop=(si == nsub - 1 and j == bn - 1))
                    koff += kt_

                    # ---------- epilogue ----------
                    rec = epi_pool.tile([128, H], F32, tag="rec")
                    nc.vector.tensor_scalar_add(out=rec[:],
                                                in0=agg_ps[:, HC:W],
                                                scalar1=SM_EPS)
                    nc.vector.reciprocal(out=rec[:], in_=rec[:])
                    aggn = epi_pool.tile([128, HC], BF16, tag="aggn")
                    nc.vector.tensor_tensor(
                        out=aggn[:].rearrange("p (h c) -> p h c", c=C),
                        in0=agg_ps[:, 0:HC].rearrange("p (h c) -> p h c", c=C),
                        in1=rec[:][:, :, None].to_broadcast([128, H, C]),
                        op=mybir.AluOpType.mult)
                    aggT_ps = ps_tr.tile([128, 2, 128], BF16, tag="trT")
                    for blk in range(2):
                        nc.tensor.transpose(
                            out=aggT_ps[:, blk, :],
                            in_=aggn[:, blk * 128:(blk + 1) * 128],
                            identity=identb[:])
                    aggT_sb = epi_pool.tile([128, 2, 128], BF16, tag="aggT_sb")
                    nc.vector.tensor_copy(out=aggT_sb[:], in_=aggT_ps[:])
                    zT_ps = ps_sT.tile([C, 128], F32, tag="sT_ps")
                    for blk in range(2):
                        nc.tensor.matmul(out=zT_ps[:],
                                         lhsT=linwt_sb[:, blk, :],
                                         rhs=aggT_sb[:, blk, :],
                                         start=(blk == 0), stop=(blk == 1))
                    # zr = relu(zT + linb2); r = zr + hT
                    zrT = epi_pool.tile([C, 128], BF16, tag="zrT")
                    nc.vector.scalar_tensor_tensor(
                        out=zrT[:], in0=zT_ps[:], scalar=linb2_sb[:],
                        in1=zeros_c[:], op0=mybir.AluOpType.add,
                        op1=mybir.AluOpType.max)
                    rT = epi_pool.tile([C, 128], BF16, tag="rT")
                    nc.vector.tensor_tensor(out=rT[:], in0=zrT[:],
                                            in1=hT_store[:, t, :],
                                            op=mybir.AluOpType.add)
                    r_ps = ps_ex.tile([128, C], BF16, tag="ex_ps")
                    nc.tensor.transpose(out=r_ps[:], in_=rT[:],
                                        identity=identb[:C, :C])
                    # LayerNorm
                    stats = ln_pool.tile([128, 6], F32, tag="stats")
                    nc.vector.bn_stats(out=stats[:], in_=r_ps[:])
                    mv = ln_pool.tile([128, 2], F32, tag="mv")
                    nc.vector.bn_aggr(out=mv[:], in_=stats[:])
                    sd = ln_pool.tile([128, 1], F32, tag="sd")
                    nc.scalar.activation(out=sd[:], in_=mv[:, 1:2],
                                         func=mybir.ActivationFunctionType.Ln,
                                         bias=eps_col[:])
                    nc.scalar.activation(out=sd[:], in_=sd[:],
                                         func=mybir.ActivationFunctionType.Exp,
                                         scale=-0.5)
                    rm = ln_pool.tile([128, C], F32, tag="rm")
                    nc.vector.scalar_tensor_tensor(
                        out=rm[:], in0=r_ps[:], scalar=mv[:, 0:1],
                        in1=lnw_rep[:], op0=mybir.AluOpType.subtract,
                        op1=mybir.AluOpType.mult)
                    out_sb = ln_pool.tile([128, C], F32, tag="out_sb")
                    nc.vector.scalar_tensor_tensor(
                        out=out_sb[:], in0=rm[:], scalar=sd[:],
                        in1=lnb_rep[:], op0=mybir.AluOpType.mult,
                        op1=mybir.AluOpType.add)
                    nc.sync.dma_start(out=out_d[t * 128:(t + 1) * 128, :],
                                      in_=out_sb[:])

    nc.finalize()
    return nc


# --------------------------------------------------------------------------
# entry point
# --------------------------------------------------------------------------

def _run(inputs, cfg):
    N, IC, C, H, NCORES = cfg["N"], cfg["IC"], cfg["C"], cfg["H"], cfg["NCORES"]
    HC = H * C
    x = np.asarray(inputs["x"], np.float32)
    meta, perms, coreinfo, xtts = _preprocess(
        x, np.asarray(inputs["edge_index"]), cfg)
    key = (tuple(sorted((k, v) for k, v in cfg.items() if k != "TRACE")),
           meta["KA"], meta["KB"])
    if key not in _PROGRAM_CACHE:
        _PROGRAM_CACHE[key] = _build_program(cfg, meta)
    nc = _PROGRAM_CACHE[key]

    ICP, KT = meta["ICP"], meta["KT"]
    ae_w = np.zeros((C, ICP), np.float32)
    ae_w[:, :IC] = np.asarray(inputs["ae_w"], np.float32)
    aewt = np.ascontiguousarray(
        ae_w.T.reshape(KT, 128, C).transpose(1, 0, 2)).astype(BFNP)

    att = np.asarray(inputs["att"], np.float32).reshape(HC)
    attT = np.zeros((128, 2, H), np.float32)
    for blk in range(2):
        for c in range(128):
            h = (blk * 128 + c) // C
            attT[c, blk, h] = att[blk * 128 + c]

    lin_w = np.asarray(inputs["lin_w"], np.float32)          # [C, HC]
    linwt = np.ascontiguousarray(
        lin_w.T.reshape(2, 128, C).transpose(1, 0, 2)).astype(BFNP)
    linb2 = (np.asarray(inputs["lin_b"], np.float32)
             + np.asarray(inputs["gat_b"], np.float32) @ lin_w.T)

    common = dict(
        aewt=aewt,
        wlt=np.ascontiguousarray(
            np.asarray(inputs["wl"], np.float32).T).astype(BFNP),
        wrt=np.ascontiguousarray(
            np.asarray(inputs["wr"], np.float32).T).astype(BFNP),
        linwt=linwt,
        attT=attT.astype(BFNP),
        aeb=np.asarray(inputs["ae_b"], np.float32).reshape(C, 1),
        linb2=linb2.reshape(C, 1),
        lnw=np.asarray(inputs["ln_w"], np.float32),
        lnb=np.asarray(inputs["ln_b"], np.float32),
    )
    in_maps = []
    for k in range(NCORES):
        ci = coreinfo[k]
        m = dict(common)
        m["xtt"] = xtts[k]
        m["idxa"] = np.ascontiguousarray(ci["idxa"])
        m["idxb"] = (np.ascontiguousarray(ci["idxb"]) if ci["idxb"].shape[1]
                     else np.zeros((128, 1), np.int16))
        m["valid"] = np.ascontiguousarray(ci["valid"])
        in_maps.append(m)

    res = bass_utils.run_bass_kernel_spmd(
        nc, in_maps, core_ids=list(range(NCORES)),
        trace=bool(cfg.get("TRACE", False)))
    NSH = meta["NSH"]
    out = np.zeros((N, C), np.float32)
    for k in range(NCORES):
        out[k * NSH + perms[k]] = res.results[k]["out"][:NSH]
    return out, res


def kernel(**inputs) -> np.ndarray:
    out, _ = _run(inputs, FULL_CFG)
    return out


# revision 3
# speedup vs baseline: 1.3005x; 1.3005x over previous
"""GATv2 encoder (nn_Encoder_83614423318750) — v2 8-core TRN2 Bass kernel.

Layout strategy (per core, SPMD; nodes block-sharded, degree-sorted):

  A : hT = (x @ ae_w.T + ae_b).T computed directly in [C, n] layout via
      PE (lhsT = ae_w chunks, rhs = x^T chunks) -> hT_store (SBUF, bf16).
      xr = h @ wr.T node-major via PE (lhsT = hT slice, rhs = wr.T).
  AG: AllGather hT shards -> hT_full [8*64, NPAD] (DRAM).
  A2: xl_full[n, hc] = h @ wl.T for ALL nodes, via PE (lhsT = hT_full
      slice, rhs = wl.T) -> DRAM bf16 (gather source).
  B : per 128-dst-node tile, per slot-subgroup (8 slots = 8 chunks of
      128 edges, chunk = slot s of all 128 dst nodes):
        * dma_gather  g   [128d, 8s, 256c]   (messages, edge-major)
        * dma_gather  gT  [128c, 2, 8s*128d] (transpose=True: score side)
        * pT = prelu(gT + xrT)               (DVE add + ScalarE Prelu)
        * sT[4h, e] = attT.T @ pT            (PE, PSUM-accumulated blocks)
        * exT = exp(sT)  (no segment-max: scores are O(5), fp32-exp safe)
        * ex[128d, s, 4h] via PE transpose;  ex *= valid (kills padding)
        * rhs = [ g * ex_bc | ex ]  [128, 260]
        * agg_ps[128, 260] += I.T @ rhs      (identity matmul: segment-sum
          of messages AND softmax denominators in one PSUM accumulation)
      epilogue: alpha-normalize by 1/ssum, z = relu(lin(agg)+lin_b+gat_b
      (folded)), r = z + h, LayerNorm -> out (fp32).

Padding slots carry valid=0 -> ex=0 -> zero contribution. Gather tables
int16 split at TSPLIT=32768 rows (A/B) as in the original kernel.
"""

import numpy as np
from contextlib import ExitStack

import ml_dtypes

import concourse.bass as bass
import concourse.bacc as bacc
import concourse.tile as tile
from concourse import mybir, bass_utils
from concourse.masks import make_identity

F32 = mybir.dt.float32
BF16 = mybir.dt.bfloat16
I16 = mybir.dt.int16
BFNP = ml_dtypes.bfloat16

FULL_CFG = dict(N=50000, IC=2000, C=64, H=4, E=800000, NCORES=8, TSPLIT=32768)

NEG_SLOPE = 0.2
LN_EPS = 1e-12
SM_EPS = 1e-9
SG = 8      # slots (=128-edge chunks) per subgroup
A2G = 4     # node-chunks per xl write group

_PROGRAM_CACHE = {}


# --------------------------------------------------------------------------
# host-side preprocessing (slot tables identical in structure to v1)
# --------------------------------------------------------------------------

def _preprocess(x, edge_index, cfg):
    N, IC, C, H, NCORES = cfg["N"], cfg["IC"], cfg["C"], cfg["H"], cfg["NCORES"]
    TSPLIT = cfg["TSPLIT"]
    NSH = N // NCORES
    NT = (NSH + 127) // 128
    NPAD = NT * 128
    NTOT = NCORES * NPAD
    ICP = ((IC + 127) // 128) * 128
    KT = ICP // 128

    src = np.asarray(edge_index[0], dtype=np.int64)
    dst = np.asarray(edge_index[1], dtype=np.int64)

    order = np.argsort(dst, kind="stable")
    src_s = src[order].astype(np.int64)
    counts = np.bincount(dst, minlength=N)
    starts = np.zeros(N, np.int64)
    starts[1:] = np.cumsum(counts)[:-1]
    deg = counts + 1  # + self loop

    perms = []
    for k in range(NCORES):
        degk = deg[k * NSH:(k + 1) * NSH]
        perms.append(np.argsort(-degk, kind="stable"))

    gmap = np.zeros(N, np.int64)
    for k in range(NCORES):
        gmap[k * NSH + perms[k]] = k * NPAD + np.arange(NSH)

    KMAXDEG = int(deg.max())
    jj = np.arange(KMAXDEG)[None, :]

    nA_all = np.zeros((NCORES, NPAD), np.int64)
    nB_all = np.zeros((NCORES, NPAD), np.int64)
    EMg_all = []
    for k in range(NCORES):
        perm = perms[k]
        vglob = k * NSH + perm
        dpn = np.zeros(NPAD, np.int64)
        dpn[:NSH] = deg[vglob]
        st = np.zeros(NPAD, np.int64)
        st[:NSH] = starts[vglob]
        vg = np.zeros(NPAD, np.int64)
        vg[:NSH] = vglob

        valid = jj < dpn[:, None]
        is_self = jj == (dpn - 1)[:, None]
        eidx = np.minimum(st[:, None] + jj, len(src_s) - 1)
        esrc = np.where(valid & ~is_self, src_s[eidx], vg[:, None])
        EMg = np.where(valid, gmap[esrc], 0)
        isA = (EMg < TSPLIT) & valid
        keys = np.where(valid, np.where(isA, 0, 1), 2)
        ordr = np.argsort(keys, axis=1, kind="stable")
        EMg_sorted = np.take_along_axis(EMg, ordr, axis=1)
        nA = isA.sum(1)
        nB = valid.sum(1) - nA
        nA_all[k], nB_all[k] = nA, nB
        EMg_all.append(EMg_sorted)

    KA = np.zeros(NT, np.int64)
    KB = np.zeros(NT, np.int64)
    for t in range(NT):
        sl = slice(t * 128, (t + 1) * 128)
        KA[t] = max(1, int(nA_all[:, sl].max()))
        KB[t] = int(nB_all[:, sl].max())
    K = KA + KB

    def pack_idx16(vals):                            # [128, Kg] -> [128, 8*Kg]
        L = vals.shape[1] * 128
        flat = vals.T.reshape(-1)                    # flat[j*128+p] = vals[p,j]
        idx16 = flat.reshape(L // 16, 16).T.astype(np.int16)
        return np.tile(idx16, (8, 1))

    coreinfo = []
    for k in range(NCORES):
        EMg_sorted = EMg_all[k]
        nA, nB = nA_all[k], nB_all[k]
        idxa_parts, idxb_parts, valid_parts = [], [], []
        for t in range(NT):
            sl = slice(t * 128, (t + 1) * 128)
            ka, kb = int(KA[t]), int(KB[t])
            em = EMg_sorted[sl]
            na = nA[sl][:, None]
            nb = nB[sl][:, None]
            ja = np.arange(ka)[None, :]
            srcA = np.where(ja < na, em[:, :ka], 0)
            idxa_parts.append(pack_idx16(srcA))
            if kb > 0:
                jb = np.arange(kb)[None, :]
                gidx = np.minimum(na + jb, EMg_sorted.shape[1] - 1)
                srcB = np.where(jb < nb,
                                np.take_along_axis(em, gidx, axis=1) - TSPLIT, 0)
                srcB = np.maximum(srcB, 0)
                idxb_parts.append(pack_idx16(srcB))
            m = np.zeros((128, ka + kb), np.float32)
            m[:, :ka][ja < na] = 1.0
            if kb > 0:
                m[:, ka:][jb < nb] = 1.0
            valid_parts.append(m)
        coreinfo.append(dict(
            idxa=np.concatenate(idxa_parts, axis=1),
            idxb=(np.concatenate(idxb_parts, axis=1) if idxb_parts
                  else np.zeros((128, 0), np.int16)),
            valid=np.concatenate(valid_parts, axis=1).astype(BFNP),
        ))

    # x shards: permuted, padded, transposed, tiled, bf16
    xtts = []
    for k in range(NCORES):
        xs = np.zeros((NPAD, ICP), np.float32)
        xs[:NSH, :IC] = x[k * NSH:(k + 1) * NSH][perms[k]]
        xtt = xs.reshape(NT, 128, KT, 128).transpose(0, 3, 2, 1)
        xtts.append(np.ascontiguousarray(xtt).astype(BFNP))

    meta = dict(NSH=NSH, NT=NT, NPAD=NPAD, NTOT=NTOT, ICP=ICP, KT=KT,
                KA=tuple(int(v) for v in KA), KB=tuple(int(v) for v in KB),
                K=tuple(int(v) for v in K))
    return meta, perms, coreinfo, xtts


# --------------------------------------------------------------------------
# device program
# --------------------------------------------------------------------------

def _build_program(cfg, meta):
    C, H, NCORES = cfg["C"], cfg["H"], cfg["NCORES"]
    HC = H * C
    NT, NPAD, NTOT = meta["NT"], meta["NPAD"], meta["NTOT"]
    TSPLIT = min(cfg["TSPLIT"], NTOT)
    KT = meta["KT"]
    KA, KB, K = meta["KA"], meta["KB"], meta["K"]
    SUMK = sum(K)
    SUMIA = sum(8 * ka for ka in KA)
    SUMIB = sum(8 * kb for kb in KB)
    W = HC + H  # 260: [amsg | ex]

    nc = bacc.Bacc("TRN2", target_bir_lowering=False, debug=False,
                   num_devices=NCORES)

    # ---- external I/O ----
    xtt = nc.dram_tensor("xtt", [NT, 128, KT, 128], BF16, kind="ExternalInput")
    aewt = nc.dram_tensor("aewt", [128, KT, C], BF16, kind="ExternalInput")
    wlt = nc.dram_tensor("wlt", [C, HC], BF16, kind="ExternalInput")
    wrt = nc.dram_tensor("wrt", [C, HC], BF16, kind="ExternalInput")
    linwt = nc.dram_tensor("linwt", [128, 2, C], BF16, kind="ExternalInput")
    attT = nc.dram_tensor("attT", [128, 2, H], BF16, kind="ExternalInput")
    aeb = nc.dram_tensor("aeb", [C, 1], F32, kind="ExternalInput")
    linb2 = nc.dram_tensor("linb2", [C, 1], F32, kind="ExternalInput")
    lnw = nc.dram_tensor("lnw", [C], F32, kind="ExternalInput")
    lnb = nc.dram_tensor("lnb", [C], F32, kind="ExternalInput")
    idxa_d = nc.dram_tensor("idxa", [128, SUMIA], I16, kind="ExternalInput")
    idxb_d = nc.dram_tensor("idxb", [128, max(SUMIB, 1)], I16,
                            kind="ExternalInput")
    valid_d = nc.dram_tensor("valid", [128, SUMK], BF16, kind="ExternalInput")
    out_d = nc.dram_tensor("out", [NPAD, C], F32, kind="ExternalOutput")

    def bc_row(t, n):  # DRAM [n] -> broadcast AP [128, n]
        return bass.AP(tensor=t[:].tensor, offset=0, ap=[[0, 128], [1, n]])

    with tile.TileContext(nc) as tc:
        with ExitStack() as ctx:
            dram = ctx.enter_context(tc.tile_pool(name="dram", bufs=1,
                                                  space="DRAM"))
            hT_shard_d = dram.tile([C, NPAD], BF16)
            hT_full = dram.tile([NCORES * C, NPAD], BF16, addr_space="Shared")
            xr_d = dram.tile([NPAD, HC], BF16)
            xl_full = dram.tile([NTOT, HC], BF16)

            # ---- persistent SBUF ----
            consts = ctx.enter_context(tc.tile_pool(name="consts", bufs=1))
            identb = consts.tile([128, 128], BF16)
            make_identity(nc, identb[:])
            aewt_sb = consts.tile([128, KT, C], BF16)
            nc.sync.dma_start(out=aewt_sb[:], in_=aewt[:])
            wlt_sb = consts.tile([C, HC], BF16)
            nc.sync.dma_start(out=wlt_sb[:], in_=wlt[:])
            wrt_sb = consts.tile([C, HC], BF16)
            nc.sync.dma_start(out=wrt_sb[:], in_=wrt[:])
            linwt_sb = consts.tile([128, 2, C], BF16)
            nc.sync.dma_start(out=linwt_sb[:], in_=linwt[:])
            attT_sb = consts.tile([128, 2, H], BF16)
            nc.sync.dma_start(out=attT_sb[:], in_=attT[:])
            aeb_sb = consts.tile([C, 1], F32)
            nc.sync.dma_start(out=aeb_sb[:], in_=aeb[:])
            linb2_sb = consts.tile([C, 1], F32)
            nc.sync.dma_start(out=linb2_sb[:], in_=linb2[:])
            lnw_rep = consts.tile([128, C], F32)
            nc.sync.dma_start(out=lnw_rep[:], in_=bc_row(lnw, C))
            lnb_rep = consts.tile([128, C], F32)
            nc.sync.dma_start(out=lnb_rep[:], in_=bc_row(lnb, C))
            eps_col = consts.tile([128, 1], F32)
            nc.vector.memset(eps_col[:], LN_EPS)
            zeros_c = consts.tile([C, 128], F32)
            nc.vector.memset(zeros_c[:], 0.0)

            hT_store = consts.tile([C, NT, 128], BF16)

            idx_arena = consts.tile([128, SUMIA + max(SUMIB, 1)], I16)
            nc.sync.dma_start(out=idx_arena[:, :SUMIA], in_=idxa_d[:])
            if SUMIB > 0:
                nc.sync.dma_start(out=idx_arena[:, SUMIA:], in_=idxb_d[:])
            valid_arena = consts.tile([128, SUMK], BF16)
            nc.sync.dma_start(out=valid_arena[:], in_=valid_d[:])

            # ================= phase A =================
            with ExitStack() as actx:
                xsl_p = actx.enter_context(tc.tile_pool(name="xsl", bufs=2))
                ps_h = actx.enter_context(
                    tc.tile_pool(name="ps_h", bufs=2, space="PSUM"))
                ps_xr = actx.enter_context(
                    tc.tile_pool(name="ps_xr", bufs=2, space="PSUM"))
                sb_a = actx.enter_context(tc.tile_pool(name="sb_a", bufs=2))

                for t in range(NT):
                    xslab = xsl_p.tile([128, KT, 128], BF16, tag="xslab")
                    nc.sync.dma_start(out=xslab[:], in_=xtt[t])
                    hT_ps = ps_h.tile([C, 128], F32, tag="hT_ps")
                    for kk in range(KT):
                        nc.tensor.matmul(out=hT_ps[:],
                                         lhsT=aewt_sb[:, kk, :],
                                         rhs=xslab[:, kk, :],
                                         start=(kk == 0), stop=(kk == KT - 1))
                    # hT = hT_ps + aeb (per-partition) -> bf16
                    aeb_b = aeb_sb[:].to_broadcast([C, 128])
                    nc.vector.tensor_tensor(out=hT_store[:, t, :],
                                            in0=hT_ps[:], in1=aeb_b,
                                            op=mybir.AluOpType.add)
                    xr_ps = ps_xr.tile([128, HC], F32, tag="xr_ps")
                    nc.tensor.matmul(out=xr_ps[:], lhsT=hT_store[:, t, :],
                                     rhs=wrt_sb[:], start=True, stop=True)
                    xr_sb = sb_a.tile([128, HC], BF16, tag="xr_sb")
                    nc.vector.tensor_copy(out=xr_sb[:], in_=xr_ps[:])
                    nc.sync.dma_start(out=xr_d[t * 128:(t + 1) * 128, :],
                                      in_=xr_sb[:])
                nc.sync.dma_start(
                    out=hT_shard_d[:],
                    in_=hT_store[:].rearrange("c t p -> c (t p)"))

            # ================= AllGather =================
            nc.gpsimd.collective_compute(
                "AllGather", mybir.AluOpType.bypass,
                ins=[hT_shard_d[:].opt()], outs=[hT_full[:].opt()],
                replica_groups=[list(range(NCORES))])

            # ================= phase A2: xl_full build =================
            with ExitStack() as actx:
                htf_p = actx.enter_context(tc.tile_pool(name="htf", bufs=2))
                ps_xl = actx.enter_context(
                    tc.tile_pool(name="ps_xl", bufs=2, space="PSUM"))
                sb_xl = actx.enter_context(tc.tile_pool(name="sb_xl", bufs=2))
                for cb in range(NCORES):
                    hTf = htf_p.tile([C, NPAD], BF16, tag="hTf")
                    nc.sync.dma_start(out=hTf[:],
                                      in_=hT_full[cb * C:(cb + 1) * C, :])
                    for g0 in range(0, NT, A2G):
                        gn = min(A2G, NT - g0)
                        xl_sb = sb_xl.tile([128, A2G, HC], BF16, tag="xl_sb")
                        for i in range(gn):
                            lc = g0 + i
                            xl_ps = ps_xl.tile([128, HC], F32, tag="xl_ps")
                            nc.tensor.matmul(
                                out=xl_ps[:],
                                lhsT=hTf[:, lc * 128:(lc + 1) * 128],
                                rhs=wlt_sb[:], start=True, stop=True)
                            nc.vector.tensor_copy(out=xl_sb[:, i, :],
                                                  in_=xl_ps[:])
                        r0 = cb * NPAD + g0 * 128
                        nc.sync.dma_start(
                            out=xl_full[r0:r0 + gn * 128, :].rearrange(
                                "(t p) c -> p t c", p=128),
                            in_=xl_sb[:, :gn, :])

            # ================= phase B =================
            with ExitStack() as bctx:
                gt_pool = bctx.enter_context(tc.tile_pool(name="gt", bufs=3))
                r_pool = bctx.enter_context(tc.tile_pool(name="rhs", bufs=3))
                ext_pool = bctx.enter_context(tc.tile_pool(name="ext", bufs=3))
                xr_pool = bctx.enter_context(tc.tile_pool(name="xrl", bufs=2))
                epi_pool = bctx.enter_context(tc.tile_pool(name="epi", bufs=2))
                ln_pool = bctx.enter_context(tc.tile_pool(name="ln", bufs=2))
                ps_sT = bctx.enter_context(
                    tc.tile_pool(name="ps_sT", bufs=1, space="PSUM"))
                ps_ex = bctx.enter_context(
                    tc.tile_pool(name="ps_ex", bufs=1, space="PSUM"))
                ps_agg = bctx.enter_context(
                    tc.tile_pool(name="ps_agg", bufs=1, space="PSUM"))
                ps_g = bctx.enter_context(
                    tc.tile_pool(name="ps_g", bufs=2, space="PSUM"))
                ps_tr = bctx.enter_context(
                    tc.tile_pool(name="ps_tr", bufs=2, space="PSUM"))

                ioffA = 0
                ioffB = SUMIA
                koff = 0
                for t in range(NT):
                    ka, kb, kt_ = KA[t], KB[t], K[t]

                    xr_t = xr_pool.tile([128, HC], BF16, tag="xr_t")
                    nc.sync.dma_start(out=xr_t[:],
                                      in_=xr_d[t * 128:(t + 1) * 128, :])
                    xrT_ps = ps_tr.tile([128, 2, 128], BF16, tag="trT")
                    for blk in range(2):
                        nc.tensor.transpose(
                            out=xrT_ps[:, blk, :],
                            in_=xr_t[:, blk * 128:(blk + 1) * 128],
                            identity=identb[:])
                    xrT_sb = xr_pool.tile([128, 2, 128], BF16, tag="xrT_sb")
                    nc.vector.tensor_copy(out=xrT_sb[:], in_=xrT_ps[:])

                    agg_ps = ps_agg.tile([128, W], F32, tag="agg_ps")

                    # subgroup list: (source-half, slot0, bn, idx column off)
                    subs = []
                    for s0 in range(0, ka, SG):
                        subs.append((0, s0, min(SG, ka - s0),
                                     ioffA + 8 * s0))
                    for s0 in range(0, kb, SG):
                        subs.append((1, ka + s0, min(SG, kb - s0),
                                     ioffB + 8 * s0))
                    ioffA += 8 * ka
                    ioffB += 8 * kb

                    chunkctr = 0
                    for si, (half, s0, bn, ioff) in enumerate(subs):
                        src = (xl_full[0:TSPLIT, :] if half == 0
                               else xl_full[TSPLIT:NTOT, :])
                        sT_ps = ps_sT.tile([H, SG * 128], F32, tag="sT_ps")
                        rhs_sb = r_pool.tile([128, SG, W], BF16, tag="rhs")
                        # half-subgroups of <=4 chunks (transpose dma_gather
                        # breaks above ~768 idxs); e-part g comes from PE
                        # transposes of gT (no second gather stream).
                        for c0 in range(0, bn, 4):
                            cn = min(4, bn - c0)
                            hne = cn * 128
                            gt_flat = gt_pool.tile([128, 2 * 4 * 128], BF16,
                                                   tag="gT")
                            gT = gt_flat[:, :2 * hne].rearrange(
                                "p (b e) -> p b e", b=2)
                            nc.gpsimd.dma_gather(
                                gT, src, idx_arena[:, ioff + 8 * c0:
                                                   ioff + 8 * (c0 + cn)],
                                hne, hne, HC, transpose=True)
                            g_ps = ps_g.tile([128, 4, HC], BF16, tag="g_ps")
                            for jj in range(cn):
                                for blk in range(2):
                                    nc.tensor.transpose(
                                        out=g_ps[:, jj,
                                                 blk * 128:(blk + 1) * 128],
                                        in_=gT[:, blk,
                                               jj * 128:(jj + 1) * 128],
                                        identity=identb[:])
                            pt_flat = ext_pool.tile([128, 2 * 4 * 128], BF16,
                                                    tag="pT")
                            pT4 = pt_flat[:, :2 * hne].rearrange(
                                "p (b k e) -> p b k e", b=2, k=cn)
                            gT4 = gt_flat[:, :2 * hne].rearrange(
                                "p (b k e) -> p b k e", b=2, k=cn)
                            xrT_b = xrT_sb[:][:, :, None, :].to_broadcast(
                                [128, 2, cn, 128])
                            nc.vector.tensor_tensor(out=pT4, in0=gT4,
                                                    in1=xrT_b,
                                                    op=mybir.AluOpType.add)
                            nc.scalar.activation(
                                out=pt_flat[:, :2 * hne],
                                in_=pt_flat[:, :2 * hne],
                                func=mybir.ActivationFunctionType.Prelu,
                                alpha=NEG_SLOPE)
                            for blk in range(2):
                                nc.tensor.matmul(
                                    out=sT_ps[:, c0 * 128:c0 * 128 + hne],
                                    lhsT=attT_sb[:, blk, :],
                                    rhs=pt_flat[:, blk * hne:
                                                blk * hne + hne],
                                    start=(blk == 0), stop=(blk == 1))
                            exT = ext_pool.tile([H, 4 * 128], BF16,
                                                tag="exT")
                            nc.scalar.activation(
                                out=exT[:, :hne],
                                in_=sT_ps[:, c0 * 128:c0 * 128 + hne],
                                func=mybir.ActivationFunctionType.Exp)
                            ex_ps = ps_ex.tile([128, 4, H], BF16,
                                               tag="ex_ps")
                            for jj in range(cn):
                                nc.tensor.transpose(
                                    out=ex_ps[:, jj, :],
                                    in_=exT[:, jj * 128:(jj + 1) * 128],
                                    identity=identb[:H, :H])
                            vb = valid_arena[:, koff + s0 + c0:
                                             koff + s0 + c0 + cn]
                            nc.vector.tensor_tensor(
                                out=rhs_sb[:, c0:c0 + cn, HC:W],
                                in0=ex_ps[:, :cn, :],
                                in1=vb[:, :, None].to_broadcast(
                                    [128, cn, H]),
                                op=mybir.AluOpType.mult)
                            g4 = g_ps[:, :cn, :].rearrange(
                                "p k (h c) -> p k h c", c=C)
                            ex_b = rhs_sb[:, c0:c0 + cn, HC:W][:, :, :, None] \
                                .to_broadcast([128, cn, H, C])
                            nc.vector.tensor_tensor(
                                out=rhs_sb[:, c0:c0 + cn, 0:HC].rearrange(
                                    "p k (h c) -> p k h c", c=C),
                                in0=g4, in1=ex_b, op=mybir.AluOpType.mult)
                            for jj in range(cn):
                                nc.tensor.matmul(
                                    out=agg_ps[:],
                                    lhsT=identb[:],
                                    rhs=rhs_sb[:, c0 + jj, :],
                                    start=(chunkctr == 0),
                                    stop=(chunkctr == kt_ - 1))
                                chunkctr += 1
                    koff += kt_

                    # ---------- epilogue ----------
                    rec = epi_pool.tile([128, H], F32, tag="rec")
                    nc.vector.tensor_scalar_add(out=rec[:],
                                                in0=agg_ps[:, HC:W],
                                                scalar1=SM_EPS)
                    nc.vector.reciprocal(out=rec[:], in_=rec[:])
                    aggn = epi_pool.tile([128, HC], BF16, tag="aggn")
                    nc.vector.tensor_tensor(
                        out=aggn[:].rearrange("p (h c) -> p h c", c=C),
                        in0=agg_ps[:, 0:HC].rearrange("p (h c) -> p h c", c=C),
                        in1=rec[:][:, :, None].to_broadcast([128, H, C]),
                        op=mybir.AluOpType.mult)
                    aggT_ps = ps_tr.tile([128, 2, 128], BF16, tag="trT")
                    for blk in range(2):
                        nc.tensor.transpose(
                            out=aggT_ps[:, blk, :],
                            in_=aggn[:, blk * 128:(blk + 1) * 128],
                            identity=identb[:])
                    aggT_sb = epi_pool.tile([128, 2, 128], BF16, tag="aggT_sb")
                    nc.vector.tensor_copy(out=aggT_sb[:], in_=aggT_ps[:])
                    zT_ps = ps_sT.tile([C, 128], F32, tag="sT_ps")
                    for blk in range(2):
                        nc.tensor.matmul(out=zT_ps[:],
                                         lhsT=linwt_sb[:, blk, :],
                                         rhs=aggT_sb[:, blk, :],
                                         start=(blk == 0), stop=(blk == 1))
                    # zr = relu(zT + linb2); r = zr + hT
                    zrT = epi_pool.tile([C, 128], BF16, tag="zrT")
                    nc.vector.scalar_tensor_tensor(
                        out=zrT[:], in0=zT_ps[:], scalar=linb2_sb[:],
                        in1=zeros_c[:], op0=mybir.AluOpType.add,
                        op1=mybir.AluOpType.max)
                    rT = epi_pool.tile([C, 128], BF16, tag="rT")
                    nc.vector.tensor_tensor(out=rT[:], in0=zrT[:],
                                            in1=hT_store[:, t, :],
                                            op=mybir.AluOpType.add)
                    r_ps = ps_ex.tile([128, C], BF16, tag="ex_ps")
                    nc.tensor.transpose(out=r_ps[:], in_=rT[:],
                                        identity=identb[:C, :C])
                    # LayerNorm
                    stats = ln_pool.tile([128, 6], F32, tag="stats")
                    nc.vector.bn_stats(out=stats[:], in_=r_ps[:])
                    mv = ln_pool.tile([128, 2], F32, tag="mv")
                    nc.vector.bn_aggr(out=mv[:], in_=stats[:])
                    sd = ln_pool.tile([128, 1], F32, tag="sd")
                    nc.scalar.activation(out=sd[:], in_=mv[:, 1:2],
                                         func=mybir.ActivationFunctionType.Ln,
                                         bias=eps_col[:])
                    nc.scalar.activation(out=sd[:], in_=sd[:],
                                         func=mybir.ActivationFunctionType.Exp,
                                         scale=-0.5)
                    rm = ln_pool.tile([128, C], F32, tag="rm")
                    nc.vector.scalar_tensor_tensor(
                        out=rm[:], in0=r_ps[:], scalar=mv[:, 0:1],
                        in1=lnw_rep[:], op0=mybir.AluOpType.subtract,
                        op1=mybir.AluOpType.mult)
                    out_sb = ln_pool.tile([128, C], F32, tag="out_sb")
                    nc.vector.scalar_tensor_tensor(
                        out=out_sb[:], in0=rm[:], scalar=sd[:],
                        in1=lnb_rep[:], op0=mybir.AluOpType.mult,
                        op1=mybir.AluOpType.add)
                    nc.sync.dma_start(out=out_d[t * 128:(t + 1) * 128, :],
                                      in_=out_sb[:])

    nc.finalize()
    return nc


# --------------------------------------------------------------------------
# entry point
# --------------------------------------------------------------------------

def _run(inputs, cfg):
    N, IC, C, H, NCORES = cfg["N"], cfg["IC"], cfg["C"], cfg["H"], cfg["NCORES"]
    HC = H * C
    x = np.asarray(inputs["x"], np.float32)
    meta, perms, coreinfo, xtts = _preprocess(
        x, np.asarray(inputs["edge_index"]), cfg)
    key = (tuple(sorted((k, v) for k, v in cfg.items() if k != "TRACE")),
           meta["KA"], meta["KB"])
    if key not in _PROGRAM_CACHE:
        _PROGRAM_CACHE[key] = _build_program(cfg, meta)
    nc = _PROGRAM_CACHE[key]

    ICP, KT = meta["ICP"], meta["KT"]
    ae_w = np.zeros((C, ICP), np.float32)
    ae_w[:, :IC] = np.asarray(inputs["ae_w"], np.float32)
    aewt = np.ascontiguousarray(
        ae_w.T.reshape(KT, 128, C).transpose(1, 0, 2)).astype(BFNP)

    att = np.asarray(inputs["att"], np.float32).reshape(HC)
    attT = np.zeros((128, 2, H), np.float32)
    for blk in range(2):
        for c in range(128):
            h = (blk * 128 + c) // C
            attT[c, blk, h] = att[blk * 128 + c]

    lin_w = np.asarray(inputs["lin_w"], np.float32)          # [C, HC]
    linwt = np.ascontiguousarray(
        lin_w.T.reshape(2, 128, C).transpose(1, 0, 2)).astype(BFNP)
    linb2 = (np.asarray(inputs["lin_b"], np.float32)
             + np.asarray(inputs["gat_b"], np.float32) @ lin_w.T)

    common = dict(
        aewt=aewt,
        wlt=np.ascontiguousarray(
            np.asarray(inputs["wl"], np.float32).T).astype(BFNP),
        wrt=np.ascontiguousarray(
            np.asarray(inputs["wr"], np.float32).T).astype(BFNP),
        linwt=linwt,
        attT=attT.astype(BFNP),
        aeb=np.asarray(inputs["ae_b"], np.float32).reshape(C, 1),
        linb2=linb2.reshape(C, 1),
        lnw=np.asarray(inputs["ln_w"], np.float32),
        lnb=np.asarray(inputs["ln_b"], np.float32),
    )
    in_maps = []
    for k in range(NCORES):
        ci = coreinfo[k]
        m = dict(common)
        m["xtt"] = xtts[k]
        m["idxa"] = np.ascontiguousarray(ci["idxa"])
        m["idxb"] = (np.ascontiguousarray(ci["idxb"]) if ci["idxb"].shape[1]
                     else np.zeros((128, 1), np.int16))
        m["valid"] = np.ascontiguousarray(ci["valid"])
        in_maps.append(m)

    res = bass_utils.run_bass_kernel_spmd(
        nc, in_maps, core_ids=list(range(NCORES)),
        trace=bool(cfg.get("TRACE", False)))
    NSH = meta["NSH"]
    out = np.zeros((N, C), np.float32)
    for k in range(NCORES):
        out[k * NSH + perms[k]] = res.results[k]["out"][:NSH]
    return out, res


def kernel(**inputs) -> np.ndarray:
    out, _ = _run(inputs, FULL_CFG)
    return out
